# revision 20
# baseline (speedup 1.0000x reference)
"""Trainium2 Bass kernel for the segmented-attention block.

~205-208us HW exec on 8 cores (baseline 278us); PE-bound: Tensor-engine
union busy ~181us vs the 177.5us bf16 roofline for this decomposition
(426k streamed matmul columns/core at 2.4GHz), plus ~25us of NEFF
handshake/queue-spin-up/drain envelope. fp8 was measured and rejected:
even V-path-only e4m3 gives 2.7% rel err vs the 2% budget.

Reference computation (per batch row b of x [B, S*D]):
    xs = x[b].reshape(S, D)
    q_s = xs[s] @ Q[s]; k_s = xs[s] @ K[s]; v_s = xs[s] @ V[s]   (per segment)
    scores[s] = dot(q_s, k_s)
    w = scores / ||scores||_2
    y[b] = sum_s w[s] * v_s            -> [E]

Math restructure 1 (score path): scores[s] = xs[s] @ (Q[s] K[s]^T) @ xs[s]^T.
x^T M x == x^T Ã x exactly for Ã = 2*tril(sym(M),-1) + diag(M), whose upper
triangle is zero; the device does only the 10 of 16 lower-tri 128x128 block
matmuls per segment (nested-prefix accumulation: 4 wide matmuls).

Math restructure 2 (normalization commutes): y = (sum_s score_s * v_s) /
||score||_2, so the device never normalizes. Each core returns the
raw-score-weighted partial u = sum_{s in local} score_s * v_s and the raw
scores; the HOST sums partials across the segment-sharded core pair and
divides by the norm. This kills the pass1/pass2 barrier: score and V
matmuls for a segment fuse into one continuous PE stream (keeps the PE
p-state at 2.4 GHz), and x streams through SBUF once (no residency).

Sharding: 4 batch groups x 2 segment halves over 8 cores. Core c handles
rows [c//2 * 1024, ...) and segments [c%2 * 16, ...): weight (Ã/V) traffic
halves vs pure data-parallel. Per (segment, row-tile) iteration:
  PE : 4 score matmuls (tri prefix) + 4 V matmuls          (~1.39us)
  DVE: rowsum STT (score) + bf16 accumulate u += v_scaled  (~0.9us)
  Act: drain v_psum * score -> bf16                         (~0.6us)
All inputs bf16; score/psum accumulation fp32; u accumulated bf16 (adds
~0.5% error, budget is 2e-2; measured total rel err 0.0064).

Scheduling notes (measured on HW):
- whole-tile loads, md+xt on the sync hwdge queue, vd+rt on the scalar
  queue: starting the PE ~3us later but never gapping beats chunked
  early starts (every PE gap also resets the 1.2->2.4GHz p-state ramp).
- last segment is phase-split (all 8 score groups, then all 8 V groups)
  and u ships per row-tile, so the final rowsum->scale->accumulate->store
  chain overlaps the remaining V matmuls instead of serializing the tail.

Self-contained: hardcodes all shapes; imports concourse from the system
install.
"""

import sys

import numpy as np
import ml_dtypes

for _p in ("/opt/trn_rl_repo",):
    if _p not in sys.path:
        sys.path.append(_p)

B, S, D, E = 4096, 32, 512, 512
NCORES = 8
GS = 2              # segment groups (core pairs share rows, split segments)
GB = NCORES // GS   # batch groups
RLOC = B // GB      # rows per core (1024)
SL = S // GS        # segments per core (16)
P = 128             # partitions
DC = D // P         # contraction chunks per segment (4)
BT = RLOC // P      # row tiles per core (8)
NBLK = DC * (DC + 1) // 2  # lower-tri 128x128 blocks of Ã per segment (10)

_BF16 = ml_dtypes.bfloat16

_nc_cache = None


def _build_bass():
    import concourse.bass as bass
    import concourse.mybir as mybir
    import concourse.tile as tile
    from concourse import bacc
    from concourse.bass import ts
    from contextlib import ExitStack

    fp32 = mybir.dt.float32
    bf16 = mybir.dt.bfloat16
    mult = mybir.AluOpType.mult
    add = mybir.AluOpType.add
    Copy = mybir.ActivationFunctionType.Copy

    nc = bacc.Bacc("TRN2", debug=False)

    xt = nc.dram_tensor("xt", [SL, D, RLOC], bf16, kind="ExternalInput")
    xr = nc.dram_tensor("xr", [RLOC, SL, D], bf16, kind="ExternalInput")
    md = nc.dram_tensor("md", [SL, P, NBLK, P], bf16, kind="ExternalInput")
    vd = nc.dram_tensor("vd", [SL, D, E], bf16, kind="ExternalInput")
    uo = nc.dram_tensor("uo", [BT, P, E], bf16, kind="ExternalOutput")
    so = nc.dram_tensor("so", [RLOC, SL], fp32, kind="ExternalOutput")

    with ExitStack() as ctx:
        tc = ctx.enter_context(tile.TileContext(nc))
        singles = ctx.enter_context(tc.tile_pool(name="singles", bufs=1))
        xpool = ctx.enter_context(tc.tile_pool(name="xseg", bufs=3))
        mpool = ctx.enter_context(tc.tile_pool(name="mmat", bufs=3))
        vpool = ctx.enter_context(tc.tile_pool(name="vmat", bufs=3))
        rpool = ctx.enter_context(tc.tile_pool(name="xrow", bufs=3))
        jpool = ctx.enter_context(tc.tile_pool(name="junk", bufs=2))
        scpool = ctx.enter_context(tc.tile_pool(name="vsc", bufs=4))
        upsum = ctx.enter_context(tc.tile_pool(name="upsum", bufs=4, space="PSUM"))
        wpsum = ctx.enter_context(tc.tile_pool(name="wpsum", bufs=3, space="PSUM"))

        scores = singles.tile([P, BT, SL], fp32)
        u_sb = singles.tile([P, BT, E], bf16)

        offs = [0, 1, 3, 6]
        for si in range(SL):
            m_sb = mpool.tile([P, NBLK, P], bf16, tag="m")
            xs_t = xpool.tile([P, DC, RLOC], bf16, tag="x")
            v_sb = vpool.tile([P, DC, E], bf16, tag="v")
            nc.sync.dma_start(out=m_sb, in_=md[si])
            nc.sync.dma_start(
                out=xs_t, in_=xt[si].rearrange("(c p) r -> p c r", p=P)
            )
            nc.scalar.dma_start(
                out=v_sb, in_=vd[si].rearrange("(c p) e -> p c e", p=P)
            )
            rt = rpool.tile([P, BT, D], bf16, tag="r")
            nc.scalar.dma_start(
                out=rt, in_=xr[:, si].rearrange("(t p) d -> p t d", p=P)
            )
            def score_group(bt):
                # U = X Ã, descending-i nested-prefix accumulation: 4 wide
                # matmuls (N=512/384/256/128) cover the lower-tri blocks.
                # skip_group_check matmuls must NOT interleave with other
                # accumulation groups in program order (corruption).
                t_ps = upsum.tile([P, D], fp32, tag="tps")
                for i in reversed(range(DC)):
                    nc.tensor.matmul(
                        t_ps[:, 0 : (i + 1) * P],
                        xs_t[:, i, ts(bt, P)],
                        m_sb[:, offs[i] : offs[i] + i + 1],
                        start=(i == DC - 1),
                        stop=(i == 0),
                        skip_group_check=True,
                    )
                junk = jpool.tile([P, D], fp32, tag="junk")
                nc.vector.scalar_tensor_tensor(
                    out=junk,
                    in0=t_ps,
                    scalar=1.0,
                    in1=rt[:, bt],
                    op0=mult,
                    op1=mult,
                    accum_out=scores[:, bt, si : si + 1],
                )

            def v_group(bt):
                # v = X V for this segment/row-tile
                v_ps = wpsum.tile([P, E], fp32, tag="vps")
                for c in range(DC):
                    nc.tensor.matmul(
                        v_ps,
                        xs_t[:, c, ts(bt, P)],
                        v_sb[:, c],
                        start=(c == 0),
                        stop=(c == DC - 1),
                    )
                if si == 0:
                    # first segment: Act writes u directly (no accumulate)
                    nc.scalar.activation(
                        out=u_sb[:, bt],
                        in_=v_ps,
                        func=Copy,
                        scale=scores[:, bt, si : si + 1],
                    )
                else:
                    # drain with raw-score scaling on the Activation engine
                    vsc = scpool.tile([P, E], bf16, tag="vsc")
                    nc.scalar.activation(
                        out=vsc,
                        in_=v_ps,
                        func=Copy,
                        scale=scores[:, bt, si : si + 1],
                    )
                    # u += v_scaled
                    nc.vector.scalar_tensor_tensor(
                        out=u_sb[:, bt],
                        in0=vsc,
                        scalar=1.0,
                        in1=u_sb[:, bt],
                        op0=mult,
                        op1=add,
                    )
                if si == SL - 1:
                    # final value of this row tile: ship it while the
                    # remaining tiles still compute
                    nc.sync.dma_start(out=uo[bt], in_=u_sb[:, bt])

            if si < SL - 1:
                for bt in range(BT):
                    score_group(bt)
                    v_group(bt)
            else:
                # last segment: all scores first, so the post-matmul tail
                # (rowsum -> scale -> accumulate -> store) of the final row
                # tiles overlaps the remaining V matmuls
                for bt in range(BT):
                    score_group(bt)
                nc.sync.dma_start(
                    out=so.rearrange("(t p) s -> p t s", p=P), in_=scores
                )
                for bt in range(BT):
                    v_group(bt)

    nc.finalize()
    return nc


def _get_nc():
    global _nc_cache
    if _nc_cache is None:
        _nc_cache = _build_bass()
    return _nc_cache


def _prep_in_maps(x, Q, K, V):
    x = np.asarray(x, dtype=np.float32)
    Qf = np.ascontiguousarray(np.asarray(Q, dtype=np.float32))
    Kf = np.ascontiguousarray(np.asarray(K, dtype=np.float32))
    M = np.matmul(Qf, Kf.transpose(0, 2, 1))  # [S, D, D]
    # x^T M x == x^T Ã x for Ã = tril(M + M^T, -1) + diag(M): fold the
    # upper triangle onto the lower so upper-tri blocks vanish.
    At = np.tril(M + M.transpose(0, 2, 1), -1)
    ii = np.arange(D)
    At[:, ii, ii] = M[:, ii, ii]
    # pack lower-tri 128x128 blocks, i-major (row-chunk i holds cols 0..i),
    # then partition-major so each SBUF partition reads one contiguous line
    blocks = []
    for i in range(DC):
        for j in range(i + 1):
            blocks.append(At[:, i * P : (i + 1) * P, j * P : (j + 1) * P])
    mb = np.ascontiguousarray(
        np.stack(blocks, axis=1).transpose(0, 2, 1, 3)  # [S, P, NBLK, P]
    ).astype(_BF16)
    vb = np.ascontiguousarray(np.asarray(V, dtype=np.float32)).astype(_BF16)
    in_maps = []
    for c in range(NCORES):
        p, h = c // GS, c % GS
        segs = slice(h * SL, (h + 1) * SL)
        xc = x[p * RLOC : (p + 1) * RLOC].reshape(RLOC, S, D)[:, segs]
        xtc = np.ascontiguousarray(xc.transpose(1, 2, 0)).astype(_BF16)  # [SL,D,R]
        xrc = np.ascontiguousarray(xc).astype(_BF16)  # [RLOC, SL, D]
        in_maps.append({"xt": xtc, "xr": xrc, "md": mb[segs], "vd": vb[segs]})
    return in_maps


def _run(in_maps, trace=False):
    from concourse.bass_utils import run_bass_kernel_spmd

    nc = _get_nc()
    res = run_bass_kernel_spmd(nc, in_maps, core_ids=list(range(NCORES)), trace=trace)
    # host-side combine: sum pair partials, normalize by raw-score L2 norm
    y = np.empty((B, E), np.float32)
    for p in range(GB):
        u = np.zeros((RLOC, E), np.float32)
        sq = np.zeros((RLOC, 1), np.float32)
        for h in range(GS):
            r = res.results[p * GS + h]
            u += r["uo"].reshape(RLOC, E).astype(np.float32)
            sc = r["so"].astype(np.float32)
            sq += np.sum(sc * sc, axis=1, keepdims=True)
        y[p * RLOC : (p + 1) * RLOC] = u / np.sqrt(sq)
    return y, res


def kernel(x=None, Q=None, K=None, V=None, **_ignored):
    in_maps = _prep_in_maps(x, Q, K, V)
    y, _ = _run(in_maps, trace=False)
    return y


def kernel_traced(x, Q, K, V):
    in_maps = _prep_in_maps(x, Q, K, V)
    return _run(in_maps, trace=True)
